# revision 1
# baseline (speedup 1.0000x reference)
"""Trainium2 Bass kernel for BinaryLinearUnit:
    y = sign(x) @ sign(w).T ; BatchNorm1d(train) ; * gamma + beta

Strategy: 2D sharding over 8 NeuronCores — 4 batch shards x 2
output-feature shards. Each core computes y.T for its [2048 batch x
2048 out-features] block with an FP8 (DoubleRow) matmul. Versus pure
data-parallel this cuts per-core HBM traffic (the dominant baseline
cost) from ~100MB to ~32MB:
  - x ships as fp8e5m2 (sign-preserving for N(0,1) values up to a
    ~6e-6 fraction that rounds to 0; adds ~2.5e-3 rel err, well under
    tolerance), K-major: 8MB/core.
  - w ships as bf16 (bf16 cast preserves sign exactly), K-major
    packed: 16MB/core.
  - y_hat ships back as fp16 (~5e-4 rel err): 8MB/core.

Signs: x-sign on DVE via one tensor_scalar (is_ge 0, sub 0.5) giving
{-0.5,+0.5} fp8 — BatchNorm cancels any constant scale of y exactly,
so +-0.5 works as well as +-1 and needs a single instruction. w-sign
on ACT (scalar.sign) giving +-1 fp8. PSUM accumulation is fp32-exact
(y/2 is a sum of +-0.5 with |y|<=4096, and even y is exact in fp16).

BN batch stats need cross-core reduction only within each group of 4
cores that shares the same output-feature shard: partial [mean, E[y^2]]
per channel are AllGathered over replica groups [[0-3],[4-7]] and
summed locally. The output tiles are processed in NSPLIT stat groups;
each group's collective is issued as soon as its matmuls finish and its
post-collective math + normalization are anchored a few tiles later,
so all BN work except the last group's overlaps the remaining matmuls.

Engine assignment: PE matmuls | ACT w-sign + sqrt + half the output
stores | DVE x-sign, bn_stats, psum->f16 copy, stats math, normalize,
x-input DMA queue | GpSimd collectives + readback | Sync w/gb DMA +
half the output stores.
"""

import numpy as np
import ml_dtypes

import concourse.bass as bass
import concourse.mybir as mybir
import concourse.tile as tile
from concourse import bacc
from concourse.bass import ts
from concourse.bass_utils import run_bass_kernel_spmd
from concourse.tile_rust import add_dep_helper

N_CORES = 8
KB_SHARD = 4            # batch shards
KO_SHARD = 2            # output-feature shards
BN_EPS = 1e-5

f32 = mybir.dt.float32
f16 = mybir.dt.float16
bf16 = mybir.dt.bfloat16
fp8 = mybir.dt.float8e4
fp8e5 = mybir.dt.float8e5


def build(B, IN, OUT, kb=KB_SHARD, ko=KO_SHARD):
    """Per-core SPMD module. Core c handles batch shard c%kb and
    out-feature shard c//kb. Shapes: x [B, IN], w [OUT, IN]."""
    Bc = B // kb            # batch rows per core
    OUTc = OUT // ko        # out features per core
    KT = IN // 128          # k tiles (contraction)
    KP = KT // 2            # fp8 DoubleRow consumes k-pairs
    OT = OUTc // 128        # output-feature tiles per core
    NB = 512                # matmul free dim / psum bank width
    BT = Bc // NB           # b tiles per core
    n_group = N_CORES // ko  # cores sharing one out-feature shard
    groups = [[g * n_group + i for i in range(n_group)] for g in range(ko)]

    # BN stat groups over the output tiles: earlier groups' collectives
    # overlap remaining matmuls. Collectives serialize on the single CC
    # stream and cost ~21us when they absorb inter-core skew, so the
    # second-to-last group ends 2 tiles (~27us) before the matmuls do —
    # hiding its latency — and the last group is small.
    # Inter-core skew (per-core throttle-window variance) makes each
    # collective ~20-25us; two tail collectives serialize on the single CC
    # stream, so exactly ONE group ends at the last tile and the
    # second-to-last group's normalize+stores hide under its collective.
    GS = [7, 7, 2] if OT == 16 else [OT - OT // 2, OT // 2]
    NSPLIT = len(GS)
    GO = [sum(GS[:q]) for q in range(NSPLIT)]

    nc = bacc.Bacc("TRN2", target_bir_lowering=False, debug=False,
                   num_devices=N_CORES)

    # Per-core external I/O (host pre-transposed, K-major):
    #   xt[k, b] = x[(c%kb)*Bc + b, k]          as fp8e5m2
    #   w2[ot, p, ks, o] = w[(c//kb)*OUTc + ot*128 + o, ks*128 + p]  bf16
    #   yt[o, b] = out[(c%kb)*Bc + b, (c//kb)*OUTc + o]  fp16
    # xt bytes are fp8e5m2(x) host-cast; declared fp8e4 so the DMA can land
    # straight in the fp8e4 sxT tile — the sign op below is pure bitwise on
    # the raw bytes, so the dtype label never matters before it runs.
    xt = nc.dram_tensor("xt", [IN, Bc], fp8, kind="ExternalInput")
    w2 = nc.dram_tensor("w2", [OT, 128, KT, 128], bf16, kind="ExternalInput")
    gb = nc.dram_tensor("gb", [128, 2, OT], f32, kind="ExternalInput")
    yt = nc.dram_tensor("yt", [OUTc, Bc], f16, kind="ExternalOutput")

    # Collective bounce buffers per stat group: [mean/4, E[y^2]/4].
    ccin = [
        nc.dram_tensor(f"ccin{q}", [128, 2 * GS[q]], f32) for q in range(NSPLIT)
    ]
    # Local (non-shared) outputs: shared-output collectives need >4-core
    # groups; the payload is tiny so the local-output path is fine.
    ccout = [
        nc.dram_tensor(f"ccout{q}", [n_group * 128, 2 * GS[q]], f32)
        for q in range(NSPLIT)
    ]

    with tile.TileContext(nc) as tc:
        with (
            tc.tile_pool(name="big", bufs=1) as big,
            # Enough half-tile bufs that a staged w DMA trigger never waits
            # on a sign to free a slot — a pool-gated trigger
            # head-of-line-blocks the whole Sync queue behind it.
            tc.tile_pool(name="ws", bufs=6) as wsp,
            tc.tile_pool(name="sw", bufs=3) as swp,
            tc.tile_pool(name="ps", bufs=2, space="PSUM") as psp,
            tc.tile_pool(name="st", bufs=2) as stp,
            tc.tile_pool(name="outp", bufs=3) as outp,
        ):
            # Standing tensors
            sxT = big.tile([128, KT, Bc], fp8)          # sign(x)/2, K-major
            yTt = big.tile([128, OT, Bc], f16)          # y.T/2 (exact in fp16)
            mvT = big.tile([128, 2, OT], f32)           # per-core [mean, var]
            gbt = big.tile([128, 2, OT], f32)           # [gamma; beta]
            scal = big.tile([128, OT], f32)             # gamma * rstd
            nbias = big.tile([128, OT], f32)            # beta - mean * scal
            epsT = big.tile([128, 1], f32)              # BN eps / 4 (ACT bias)
            nc.vector.memset(epsT[:], BN_EPS / 4.0)
            grTs = [None] * NSPLIT                      # gathered stats tiles

            def w_chain(ot):
                # two half-K chunks for finer DMA/ACT pipelining
                swt = swp.tile([128, KT, 128], fp8, tag="swt", name="swt")
                hk = KT // 2
                for h in range(2):
                    wst = wsp.tile([128, hk, 128], bf16, tag="wst", name="wst")
                    nc.sync.dma_start(
                        out=wst[:], in_=w2[ot, :, h * hk : (h + 1) * hk, :]
                    )
                    nc.scalar.sign(swt[:, h * hk : (h + 1) * hk, :], wst[:])
                return swt

            def alloc_psums():
                return [
                    psp.tile([128, NB], f32, tag=f"ps{bt}", name=f"psum{bt}")
                    for bt in range(BT)
                ]

            def mm_mms(swt, psums):
                # kp-outer: each stationary load is reused across BT b-tiles;
                # also consumes the x k-pairs progressively during startup.
                for kp in range(KP):
                    for bt in range(BT):
                        nc.tensor.matmul(
                            psums[bt][:],
                            lhsT=swt[:, 2 * kp : 2 * kp + 2, :],
                            rhs=sxT[:, 2 * kp : 2 * kp + 2, ts(bt, NB)],
                            start=(kp == 0),
                            stop=(kp == KP - 1),
                            perf_mode=mybir.MatmulPerfMode.DoubleRow,
                        )

            def mm_fused01(swts):
                # Fused first pass over ot0+ot1, interleaved kp-major across
                # both psum generations: during the ~26us the x stream takes
                # to arrive, every landed k-pair feeds 8 matmuls instead of
                # 4, so the PE trails the DMA instead of stalling after it.
                pss = [alloc_psums() for _ in range(2)]
                for kp in range(KP):
                    for o in range(2):
                        for bt in range(BT):
                            nc.tensor.matmul(
                                pss[o][bt][:],
                                lhsT=swts[o][:, 2 * kp : 2 * kp + 2, :],
                                rhs=sxT[:, 2 * kp : 2 * kp + 2, ts(bt, NB)],
                                start=(kp == 0),
                                stop=(kp == KP - 1),
                                perf_mode=mybir.MatmulPerfMode.DoubleRow,
                            )
                return pss

            def mm_drain(ot, psums):
                # Drain PSUM on DVE. (Tried ACT: the PE lost ~25us of
                # active time — see transcript notes.) All bn_stats before
                # all copies: the aggr -> partial-stats -> collective chain
                # is the tail's critical path, and PSUM reuse has a full
                # tile of slack so the later bank release is free.
                st6 = stp.tile([128, BT, 6], f32, tag="st6", name="st6", bufs=4)
                for bt in range(BT):
                    nc.vector.bn_stats(st6[:, bt, :], psums[bt][:])
                aggr = nc.vector.bn_aggr(mvT[:, :, ot], st6[:])
                for bt in range(BT):
                    nc.vector.tensor_copy(yTt[:, ot, ts(bt, NB)], psums[bt][:])
                return aggr

            def mm_tile(ot, swt):
                psums = alloc_psums()
                mm_mms(swt, psums)
                return mm_drain(ot, psums)

            def stats_pre(q):
                """Per-core partial stats -> AllGather, right after group q's
                matmuls."""
                o0, HOT = GO[q], GS[q]
                osl = slice(o0, o0 + HOT)
                arT = stp.tile([128, 2, HOT], f32, tag="arT", name="arT")
                tmp = stp.tile([128, HOT], f32, tag="tmp_ar", name="tmp_ar")
                nc.vector.tensor_scalar_mul(arT[:, 0, :], mvT[:, 0, osl], 1.0 / n_group)
                nc.vector.tensor_mul(tmp[:], mvT[:, 0, osl], mvT[:, 0, osl])
                nc.vector.tensor_add(tmp[:], tmp[:], mvT[:, 1, osl])
                nc.vector.tensor_scalar_mul(arT[:, 1, :], tmp[:], 1.0 / n_group)
                nc.sync.dma_start(out=ccin[q][:], in_=arT[:])
                nc.gpsimd.collective_compute(
                    "AllGather",
                    mybir.AluOpType.bypass,
                    replica_groups=groups,
                    ins=[ccin[q][:]],
                    outs=[ccout[q][:]],
                )
                grA = big.tile([128, n_group, 2, HOT], f32, name=f"grA{q}")
                # SWDGE readback keeps the Sync HWDGE queue free for the
                # next group's weight loads (queues are in-order) — except
                # the last group, where Sync is idle and HWDGE is ~3us
                # faster than the SWDGE path, straight into the tail.
                eng = nc.sync if q == NSPLIT - 1 else nc.gpsimd
                eng.dma_start(
                    out=grA[:],
                    in_=ccout[q][:].rearrange("(r p) j -> p r j", p=128),
                )
                grTs[q] = grA

            def stats_post(q, anchor=None):
                """Global stats -> scale/bias for group q (anchored a few
                tiles after its collective was issued)."""
                o0, HOT = GO[q], GS[q]
                osl = slice(o0, o0 + HOT)
                grA = grTs[q]
                grT = stp.tile([128, 2, HOT], f32, tag="grT", name="grT")
                first = nc.vector.tensor_reduce(
                    grT[:],
                    grA[:].rearrange("p r two h -> p two h r"),
                    axis=mybir.AxisListType.X,
                    op=mybir.AluOpType.add,
                )
                if anchor is not None:
                    # The scheduler's cost model doesn't know collective
                    # latency; without this ordering edge it hoists the
                    # post-collective math ahead of the running group's PSUM
                    # drains on the in-order DVE queue, stalling the PE.
                    add_dep_helper(first.ins, anchor.ins, sync=False,
                                   reason="post-AR math after current group")
                gmean = grT[:, 0, :]
                gvar = stp.tile([128, HOT], f32, tag="gvar", name="gvar")
                nc.vector.tensor_mul(gvar[:], gmean, gmean)
                nc.vector.tensor_sub(gvar[:], grT[:, 1, :], gvar[:])
                # sqrt with eps folded into the ACT bias + plain reciprocal,
                # no Newton refine: the approx error is far inside the 2e-2
                # gate and this chain is the exposed post-collective tail.
                sq = stp.tile([128, HOT], f32, tag="sq", name="sq")
                nc.scalar.activation(sq[:], gvar[:],
                                     mybir.ActivationFunctionType.Sqrt,
                                     epsT[:], 1.0, 0.0)
                r = stp.tile([128, HOT], f32, tag="r", name="rstd")
                nc.vector.reciprocal(r[:], sq[:])
                t2 = stp.tile([128, HOT], f32, tag="t2", name="t2")
                nc.vector.tensor_mul(scal[:, osl], gbt[:, 0, osl], r[:])
                nc.vector.tensor_mul(t2[:], gmean, scal[:, osl])
                nc.vector.tensor_sub(nbias[:, osl], gbt[:, 1, osl], t2[:])

            def norm_group(q):
                # DVE mul-add in fp16. (Tried GPSIMD to keep DVE clear: its
                # 2.1us/op — 2.4x DVE — serialized the tail, and collective
                # triggers queue behind it on the in-order gpsimd queue. The
                # mid-run PE slowdown it was meant to fix turned out to be
                # hardware clock throttling, not DVE contention.)
                # Stores alternate between the ACT and Sync HWDGE queues.
                for ot in range(GO[q], GO[q] + GS[q]):
                    ob = outp.tile([128, Bc], f16, tag="ob", name="ob")
                    nc.vector.tensor_scalar(
                        ob[:],
                        yTt[:, ot, :],
                        scal[:, ot : ot + 1],
                        nbias[:, ot : ot + 1],
                        op0=mybir.AluOpType.mult,
                        op1=mybir.AluOpType.add,
                    )
                    eng = nc.scalar if ot % 2 else nc.sync
                    eng.dma_start(out=yt[ts(ot, 128), :], in_=ob[:])

            # ---- emission order == scheduling priority ----
            # x lands by DMA straight in the standing sxT tile (no staging
            # pool: pool-gated triggers would head-of-line-block a DMA
            # queue), one chunk per k-PAIR (the DoubleRow consumption unit),
            # consumed progressively by the kp-outer matmuls of the fused
            # first pass. Even chunks ride the ACT HWDGE queue (triggered
            # before any w-sign lands there), odd chunks ride Sync behind
            # the first two w tiles — two queues so neither w nor x
            # head-of-line-blocks the other. Sign is a single in-place DVE
            # bitwise op on uint16 views — two packed fp8 bytes per element
            # at 2x DVE rate (0.69us/chunk measured), exact even for -0.0:
            #   fp8e4(+-0.5) = 0x30 | (fp8e5m2_byte & 0x80), per byte.
            # (GPSIMD was tried for half the sign chain: 64us per chunk,
            # 26x slower than DVE.)
            # ot=0/1 weights first, in quarter-K pieces with the kp0 pieces
            # and the x kp0 chunk leading: the first matmul needs only
            # 0.25MB of w0 + 0.5MB of x, so it issues at ~11us instead of
            # waiting for two full 1MB tiles to land and sign (~19us).
            # Everything stays on the Sync queue: x racing w on a separate
            # queue delays w0's transfer by ~15us on the shared wire
            # (tried; first matmul slipped to 30us).
            def x_dma(kp):
                nc.sync.dma_start(
                    out=sxT[:, 2 * kp : 2 * kp + 2, :],
                    in_=xt[ts(kp, 256), :].rearrange("(f p) b -> p f b", p=128),
                )

            def w_piece(swt, ot, h, ck):
                wst = wsp.tile([128, ck, 128], bf16, tag="wst0", name="wst0",
                               bufs=8)
                nc.sync.dma_start(
                    out=wst[:], in_=w2[ot, :, h * ck : (h + 1) * ck, :]
                )
                nc.scalar.sign(swt[:, h * ck : (h + 1) * ck, :], wst[:])

            ck4 = KT // 4
            swt0 = swp.tile([128, KT, 128], fp8, tag="swt", name="swt")
            swt1 = swp.tile([128, KT, 128], fp8, tag="swt", name="swt")
            w_piece(swt0, 0, 0, ck4)
            w_piece(swt1, 1, 0, ck4)
            x_dma(0)
            for h in range(1, 4):
                w_piece(swt0, 0, h, ck4)
                w_piece(swt1, 1, h, ck4)
            for kp in range(1, KP):
                x_dma(kp)
            for kp in range(KP):
                s16 = sxT[:, 2 * kp : 2 * kp + 2, :].bitcast(mybir.dt.uint16)
                nc.vector.tensor_scalar(
                    s16, s16, 0x8080, 0x3030,
                    op0=mybir.AluOpType.bitwise_and,
                    op1=mybir.AluOpType.bitwise_or,
                )

            nc.sync.dma_start(out=gbt[:], in_=gb[:])

            # post/norm for group q anchored late enough that its collective
            # has certainly completed, early enough to overlap matmuls.
            # post(q) anchors ~2 tiles after its collective fires (skew+op
            # is ~22us ~= 1.5 tiles). The second-to-last group's post runs
            # at ot14 (its collective is long done) but its NORM+stores are
            # deferred past ot15's drains and the last collective's trigger:
            # 7 tiles of DVE normalize ahead of those drains would delay the
            # trigger by ~6us of pure tail (and ahead of pending PSUM copies
            # they stall the PE — seen as 9us matmul waits).
            post_at, norm_at = {}, {}
            for q in range(1, NSPLIT):
                off = 2 if q == 1 else 0
                a = min(GO[q] + off, OT - 1)
                post_at.setdefault(a, []).append(q - 1)
                norm_at.setdefault(a if q == 1 else OT - 1, []).append(q - 1)
            post_at.setdefault(OT - 1, []).append(NSPLIT - 1)
            norm_at.setdefault(OT - 1, []).append(NSPLIT - 1)

            assert GS[0] >= 3, "fused ot0/ot1 pass assumes both in group 0"
            pss01 = mm_fused01([swt0, swt1])
            swt_next = w_chain(2)
            aggrs = [mm_drain(0, pss01[0]), mm_drain(1, pss01[1])]

            for q in range(NSPLIT):
                for ot in range(max(GO[q], 2), GO[q] + GS[q]):
                    swt = swt_next
                    if ot + 1 < OT:
                        swt_next = w_chain(ot + 1)
                    aggrs.append(mm_tile(ot, swt))
                    if ot == GO[q] + GS[q] - 1:
                        stats_pre(q)
                    # the last group's post waits ~25us on its collective
                    # readback and the DVE queue is in-order, so everything
                    # that can run now must be emitted ahead of it
                    last = NSPLIT - 1
                    for pq in post_at.get(ot, []):
                        if pq != last:
                            stats_post(pq, anchor=aggrs[ot])
                    for nq in norm_at.get(ot, []):
                        if nq != last:
                            norm_group(nq)
                    if last in post_at.get(ot, []):
                        stats_post(last, anchor=aggrs[ot])
                    if last in norm_at.get(ot, []):
                        norm_group(last)

    nc.finalize()
    return nc


def shard_inputs(x, w, gamma, beta, kb=KB_SHARD, ko=KO_SHARD):
    B, IN = x.shape
    OUT = w.shape[0]
    Bc = B // kb
    OUTc = OUT // ko
    KT, OT = IN // 128, OUTc // 128
    xts = []
    for ib in range(kb):
        # fp8e5m2 keeps the sign for all but a ~6e-6 fraction of N(0,1)
        # values; viewed as e4m3 only as a dtype label for the DMA (the
        # kernel signs the raw bytes bitwise before any arithmetic).
        xts.append(np.ascontiguousarray(
            x[ib * Bc : (ib + 1) * Bc].T.astype(ml_dtypes.float8_e5m2)
        ).view(ml_dtypes.float8_e4m3))
    wgs = []
    for io in range(ko):
        ws = w[io * OUTc : (io + 1) * OUTc]
        w2 = np.ascontiguousarray(
            ws.reshape(OT, 128, KT, 128).transpose(0, 3, 2, 1)
            .astype(ml_dtypes.bfloat16)
        )
        gbp = np.ascontiguousarray(np.stack(
            [gamma[io * OUTc : (io + 1) * OUTc].reshape(OT, 128).T,
             beta[io * OUTc : (io + 1) * OUTc].reshape(OT, 128).T],
            axis=1,
        )).astype(np.float32)
        wgs.append((w2, gbp))
    in_maps = []
    for c in range(kb * ko):
        io, ib = c // kb, c % kb
        in_maps.append({"xt": xts[ib], "w2": wgs[io][0], "gb": wgs[io][1]})
    return in_maps


_NC_CACHE = {}


def kernel(x, w, gamma, beta):
    x = np.asarray(x)
    w = np.asarray(w)
    gamma = np.asarray(gamma)
    beta = np.asarray(beta)
    B, IN = x.shape
    OUT = w.shape[0]

    key = (B, IN, OUT)
    if key not in _NC_CACHE:
        _NC_CACHE[key] = build(B, IN, OUT)
    nc = _NC_CACHE[key]

    in_maps = shard_inputs(x, w, gamma, beta)
    res = run_bass_kernel_spmd(nc, in_maps, list(range(N_CORES)))
    Bc, OUTc = B // KB_SHARD, OUT // KO_SHARD
    out = np.empty((B, OUT), np.float32)
    for c in range(N_CORES):
        io, ib = c // KB_SHARD, c % KB_SHARD
        out[ib * Bc : (ib + 1) * Bc, io * OUTc : (io + 1) * OUTc] = (
            res.results[c]["yt"].T.astype(np.float32)
        )
    return out


if __name__ == "__main__":
    rng = np.random.default_rng(0)
    B, IN, OUT = 8192, 4096, 4096
    x = rng.standard_normal((B, IN)).astype(np.float32)
    w = rng.standard_normal((OUT, IN)).astype(np.float32)
    gamma = np.ones(OUT, np.float32)
    beta = np.zeros(OUT, np.float32)
    out = kernel(x, w, gamma, beta)
    print(out.shape, out.dtype)



# revision 9
# speedup vs baseline: 1.1984x; 1.1984x over previous
"""Trainium2 Bass kernel for BinaryLinearUnit:
    y = sign(x) @ sign(w).T ; BatchNorm1d(train) ; * gamma + beta

Strategy: 2D sharding over 8 NeuronCores — 4 batch shards x 2
output-feature shards. Each core computes y.T for its [2048 batch x
2048 out-features] block with an FP8 (DoubleRow) matmul.

Signs are precomputed on the host (sharding-time byte maps, exact):
  - x ships as +-0.5 fp8e4m3 bytes (0x30/0xB0), K-major: 8MB/core.
    BatchNorm cancels any constant scale of y, so +-0.5 == +-1.
  - w ships as +-1 fp8e4m3 bytes (0x38/0xB8), K-major packed: 8MB/core.
  - y_hat ships back as fp16 (~5e-4 rel err): 8MB/core.
PSUM accumulation is fp32-exact (y/2 is a sum of +-0.5 with |y|<=4096,
and y/2 is exact in fp16). No sign ops on device at all — ACT and DVE
are free for BN work, and the first matmul is gated only by the first
w/x DMA chunks.

BN batch stats need cross-core reduction only within each group of 4
cores that shares the same output-feature shard (logical groups
[0-3], [4-7] — XOR-cosets). Instead of collective_compute AllGathers
(~31us each on the CC stream, measured), partial [mean, E[y^2]] tiles
are exchanged with remote_dma_broadcast: each core sends its partial
to peer c^k which lands in slot k of the peer's SBUF landing tile
(XOR symmetry makes the same static APs correct on every core), with
a monotonic-semaphore bump on arrival. Exchange latency is a few us,
so the post-matmul tail is bn_stats + exchange + normalize (~15us)
instead of a ~45us exposed collective. Descriptors are pre-generated
on GpSimd well before each group ends; only trigger_dma is on the
critical path (the Tile-managed count=None path carries the source
read deps).

The output tiles are processed in NSPLIT stat groups; earlier groups'
exchanges + post math + normalization all overlap remaining matmuls.
The PE span is throttle-bound (GPIO 13/16 duty after ~40us — board
power limit, measured via ntff ham records), so the only other levers
are the startup to first matmul and the tail.

Engine assignment: PE matmuls | ACT x-input DMA queue + sqrt + half
the output stores | DVE bn_stats, psum->f16 copy, stats math, rdma
waits, normalize | GpSimd rdma desc-gen + triggers | Sync w/gb DMA +
half the output stores.
"""

import numpy as np
import ml_dtypes

import concourse.bass as bass
import concourse.mybir as mybir
import concourse.tile as tile
import concourse.bass_interp as bass_interp
from concourse import bacc
from concourse.bass import ts, create_sync_update
from concourse.bass_utils import run_bass_kernel_spmd
from concourse.tile_rust import add_dep_helper

N_CORES = 8
KB_SHARD = 4            # batch shards
KO_SHARD = 2            # output-feature shards
BN_EPS = 1e-5

f32 = mybir.dt.float32
f16 = mybir.dt.float16
fp8 = mybir.dt.float8e4

# The Tile scheduling pass runs a single-core no-exec sim in which
# remote-DMA sem increments from peer cores never arrive, so a wait on
# them deadlocks the scheduler. Seed those sems with a huge value in
# the scheduling pass only; on HW the real wait still blocks until the
# peers' writes land.
_SEED_SEMS: list = []
_orig_simulate = bass_interp.CoreSim.simulate


def _sim_with_seed(self, *a, **k):
    if self.is_scheduling_pass():
        for h in _SEED_SEMS:
            self.update_semaphore(
                create_sync_update(h, 1 << 20, skip_validation=True)
            )
    return _orig_simulate(self, *a, **k)


bass_interp.CoreSim.simulate = _sim_with_seed


def build(B, IN, OUT, kb=KB_SHARD, ko=KO_SHARD):
    """Per-core SPMD module. Core c handles batch shard c%kb and
    out-feature shard c//kb. Shapes: x [B, IN], w [OUT, IN]."""
    Bc = B // kb            # batch rows per core
    OUTc = OUT // ko        # out features per core
    KT = IN // 128          # k tiles (contraction)
    KP = KT // 2            # fp8 DoubleRow consumes k-pairs
    OT = OUTc // 128        # output-feature tiles per core
    NB = 512                # matmul free dim / psum bank width
    BT = Bc // NB           # b tiles per core
    n_group = N_CORES // ko  # cores sharing one out-feature shard

    # Stat groups: each group's exchange (a few us) + post + normalize
    # overlap the remaining matmuls; only the last group's chain is an
    # exposed tail, so it is small.
    GS = [7, 7, 2] if OT == 16 else [OT - OT // 2, OT // 2]
    NSPLIT = len(GS)
    GO = [sum(GS[:q]) for q in range(NSPLIT)]

    nc = bacc.Bacc("TRN2", target_bir_lowering=False, debug=False,
                   num_devices=N_CORES, monotonic_sem_count=1)

    # Per-core external I/O (host pre-transposed, K-major, pre-signed):
    #   xt[k, b] = sign(x[(c%kb)*Bc + b, k]) * 0.5          fp8e4m3
    #   w2[ot, p, ks, o] = sign(w[(c//kb)*OUTc + ot*128 + o, ks*128 + p])
    #   yt[o, b] = out[(c%kb)*Bc + b, (c//kb)*OUTc + o]     fp16
    xt = nc.dram_tensor("xt", [IN, Bc], fp8, kind="ExternalInput")
    w2 = nc.dram_tensor("w2", [OT, 128, KT, 128], fp8, kind="ExternalInput")
    gb = nc.dram_tensor("gb", [128, 2, OT], f32, kind="ExternalInput")
    yt = nc.dram_tensor("yt", [OUTc, Bc], f16, kind="ExternalOutput")

    # Dummy 8-core collective: its presence makes the runtime build the
    # global comm and align core launches (without any collective the
    # cores free-run after their own input staging, ~1.6ms apart —
    # measured: peers' rdma arrived in ms-late bursts). Nothing consumes
    # its output; it runs on the CC stream concurrent with startup DMA.
    ccd_in = nc.dram_tensor("ccd_in", [128, 1], f32)
    ccd_out = nc.dram_tensor("ccd_out", [N_CORES * 128, 1], f32)

    # Monotonic sem bumped by peers' remote writes (+2 per peer per
    # group: 8 slots -> 16//8 increments per real dest). Same sem num
    # on every core (SPMD). lsem is the send-side release sem (unused:
    # source tiles are standing, never rewritten).
    rsem = nc.monotonic_semaphore(0).sem()
    _SEED_SEMS.clear()
    _SEED_SEMS.append(rsem)
    lsem = nc.alloc_semaphore("rdma_local")

    with tile.TileContext(nc) as tc:
        with (
            tc.tile_pool(name="big", bufs=1) as big,
            tc.tile_pool(name="sw", bufs=3) as swp,
            tc.tile_pool(name="ps", bufs=2, space="PSUM") as psp,
            tc.tile_pool(name="st", bufs=2) as stp,
            tc.tile_pool(name="outp", bufs=3) as outp,
        ):
            # Standing tensors
            sxT = big.tile([128, KT, Bc], fp8)          # sign(x)/2, K-major
            yTt = big.tile([128, OT, Bc], f16)          # y.T/2 (exact in fp16)
            mvT = big.tile([128, 2, OT], f32)           # per-core [mean, var]
            gbt = big.tile([128, 2, OT], f32)           # [gamma; beta]
            scal = big.tile([128, OT], f32)             # gamma * rstd
            nbias = big.tile([128, OT], f32)            # beta - mean * scal
            epsT = big.tile([128, 1], f32)              # BN eps / 4 (ACT bias)
            nc.vector.memset(epsT[:], BN_EPS / 4.0)
            # Exchange tiles (standing: remote reads/writes outlive any
            # pool-recycle window Tile could reason about).
            arTs = [big.tile([128, 2, GS[q]], f32, name=f"arT{q}")
                    for q in range(NSPLIT)]
            # grA slot 0 = own partial, slot k = from logical peer c^k.
            grAs = [big.tile([128, n_group, 2, GS[q]], f32, name=f"grA{q}")
                    for q in range(NSPLIT)]

            def rdma_prep(q):
                # Desc-gen for group q's three peer sends, emitted well
                # before the group ends so only trigger_dma is on the
                # critical path. Slot k of the receiver gets the sender
                # at XOR-distance k (same static APs on every core).
                for k in range(1, n_group):
                    rd = [None] * 8
                    rd[k] = (0, k)
                    nc.gpsimd.remote_dma_broadcast(
                        out_ap=grAs[q][:, k, :, :], in_ap=arTs[q][:],
                        remote_sem=rsem, local_sem=lsem, rdests=rd,
                    )

            def alloc_psums():
                return [
                    psp.tile([128, NB], f32, tag=f"ps{bt}", name=f"psum{bt}")
                    for bt in range(BT)
                ]

            def mm_mms(swt, psums):
                # kp-outer: each stationary load is reused across BT b-tiles;
                # also consumes the x k-pairs progressively during startup.
                for kp in range(KP):
                    for bt in range(BT):
                        nc.tensor.matmul(
                            psums[bt][:],
                            lhsT=swt[:, 2 * kp : 2 * kp + 2, :],
                            rhs=sxT[:, 2 * kp : 2 * kp + 2, ts(bt, NB)],
                            start=(kp == 0),
                            stop=(kp == KP - 1),
                            perf_mode=mybir.MatmulPerfMode.DoubleRow,
                        )

            def mm_fused01(swts):
                # Fused first pass over ot0+ot1, interleaved kp-major across
                # both psum generations: while the x stream arrives, every
                # landed k-pair feeds 8 matmuls instead of 4, so the PE
                # trails the DMA instead of stalling after it.
                pss = [alloc_psums() for _ in range(2)]
                for kp in range(KP):
                    for o in range(2):
                        for bt in range(BT):
                            nc.tensor.matmul(
                                pss[o][bt][:],
                                lhsT=swts[o][:, 2 * kp : 2 * kp + 2, :],
                                rhs=sxT[:, 2 * kp : 2 * kp + 2, ts(bt, NB)],
                                start=(kp == 0),
                                stop=(kp == KP - 1),
                                perf_mode=mybir.MatmulPerfMode.DoubleRow,
                            )
                return pss

            def mm_drain(ot, psums, defer_casts=False):
                # Drain PSUM on DVE. All bn_stats before all copies: the
                # aggr -> partial-stats -> exchange chain is the tail's
                # critical path. For the very last tile the casts are
                # deferred past the exchange trigger (no matmuls need
                # those banks afterwards).
                st6 = stp.tile([128, BT, 6], f32, tag="st6", name="st6", bufs=4)
                for bt in range(BT):
                    nc.vector.bn_stats(st6[:, bt, :], psums[bt][:])
                aggr = nc.vector.bn_aggr(mvT[:, :, ot], st6[:])
                if not defer_casts:
                    for bt in range(BT):
                        nc.vector.tensor_copy(
                            yTt[:, ot, ts(bt, NB)], psums[bt][:]
                        )
                return aggr, psums

            def mm_tile(ot, swt, defer_casts=False):
                psums = alloc_psums()
                mm_mms(swt, psums)
                return mm_drain(ot, psums, defer_casts=defer_casts)

            def stats_pre(q):
                """Partial [mean/4, E[y^2]/4] -> own slot + trigger the
                pre-generated peer sends, right after group q's matmuls."""
                o0, HOT = GO[q], GS[q]
                osl = slice(o0, o0 + HOT)
                arT = arTs[q]
                tmp = stp.tile([128, HOT], f32, tag="tmp_ar", name="tmp_ar")
                w0 = nc.vector.tensor_scalar_mul(
                    arT[:, 0, :], mvT[:, 0, osl], 1.0 / n_group
                )
                nc.vector.tensor_mul(tmp[:], mvT[:, 0, osl], mvT[:, 0, osl])
                nc.vector.tensor_add(tmp[:], tmp[:], mvT[:, 1, osl])
                w1 = nc.vector.tensor_scalar_mul(
                    arT[:, 1, :], tmp[:], 1.0 / n_group
                )
                own = nc.vector.tensor_copy(grAs[q][:, 0, :, :], arT[:])
                trig = nc.gpsimd.trigger_dma(count=None)
                # The preps were desc-generated long before arT exists, so
                # Tile has no producer to defer the source read against —
                # wire the data dependency onto the trigger explicitly
                # (without it the sends fire at ~20us with garbage).
                add_dep_helper(trig.ins, w0.ins, sync=True,
                               reason="rdma trigger after arT mean write")
                add_dep_helper(trig.ins, w1.ins, sync=True,
                               reason="rdma trigger after arT sumsq write")
                return own

            def stats_post(q, anchor=None, pre_deps=()):
                """Wait for the three peers' writes, then global stats ->
                scale/bias for group q. The wait head-of-line-blocks the
                in-order DVE queue, so everything that can run now must be
                ordered ahead of it (pre_deps)."""
                o0, HOT = GO[q], GS[q]
                osl = slice(o0, o0 + HOT)
                w = nc.vector.wait_ge(rsem, 2 * (n_group - 1) * (q + 1))
                if anchor is not None:
                    add_dep_helper(w.ins, anchor.ins, sync=False,
                                   reason="rdma wait after local work")
                for d in pre_deps:
                    add_dep_helper(w.ins, d.ins, sync=False,
                                   reason="rdma wait after local work")
                grT = stp.tile([128, 2, HOT], f32, tag="grT", name="grT")
                first = nc.vector.tensor_reduce(
                    grT[:],
                    grAs[q][:].rearrange("p r two h -> p two h r"),
                    axis=mybir.AxisListType.X,
                    op=mybir.AluOpType.add,
                )
                add_dep_helper(first.ins, w.ins, sync=False,
                               reason="reduce after rdma wait")
                gmean = grT[:, 0, :]
                gvar = stp.tile([128, HOT], f32, tag="gvar", name="gvar")
                nc.vector.tensor_mul(gvar[:], gmean, gmean)
                nc.vector.tensor_sub(gvar[:], grT[:, 1, :], gvar[:])
                # sqrt with eps folded into the ACT bias + plain reciprocal,
                # no Newton refine: the approx error is far inside the 2e-2
                # gate and this chain is the exposed post-exchange tail.
                sq = stp.tile([128, HOT], f32, tag="sq", name="sq")
                nc.scalar.activation(sq[:], gvar[:],
                                     mybir.ActivationFunctionType.Sqrt,
                                     epsT[:], 1.0, 0.0)
                r = stp.tile([128, HOT], f32, tag="r", name="rstd")
                nc.vector.reciprocal(r[:], sq[:])
                t2 = stp.tile([128, HOT], f32, tag="t2", name="t2")
                nc.vector.tensor_mul(scal[:, osl], gbt[:, 0, osl], r[:])
                nc.vector.tensor_mul(t2[:], gmean, scal[:, osl])
                nc.vector.tensor_sub(nbias[:, osl], gbt[:, 1, osl], t2[:])

            def norm_group(q):
                # DVE mul-add in fp16; stores alternate ACT/Sync HWDGE.
                last = None
                for ot in range(GO[q], GO[q] + GS[q]):
                    ob = outp.tile([128, Bc], f16, tag="ob", name="ob")
                    last = nc.vector.tensor_scalar(
                        ob[:],
                        yTt[:, ot, :],
                        scal[:, ot : ot + 1],
                        nbias[:, ot : ot + 1],
                        op0=mybir.AluOpType.mult,
                        op1=mybir.AluOpType.add,
                    )
                    eng = nc.scalar if ot % 2 else nc.sync
                    eng.dma_start(out=yt[ts(ot, 128), :], in_=ob[:])
                return last

            # ---- emission order == scheduling priority ----
            # x lands by DMA straight in the standing sxT tile, one chunk
            # per k-PAIR (the DoubleRow consumption unit), on the ACT HWDGE
            # queue; w rides Sync — two queues so neither head-of-line-
            # blocks the other. First matmul needs only the kp0 quarter of
            # w0/w1 plus the x kp0 chunk, so those lead.
            def x_dma(kp):
                nc.scalar.dma_start(
                    out=sxT[:, 2 * kp : 2 * kp + 2, :],
                    in_=xt[ts(kp, 256), :].rearrange("(f p) b -> p f b", p=128),
                )

            def w_piece(swt, ot, h, ck):
                nc.sync.dma_start(
                    out=swt[:, h * ck : (h + 1) * ck, :],
                    in_=w2[ot, :, h * ck : (h + 1) * ck, :],
                )

            def w_dma(ot):
                swt = swp.tile([128, KT, 128], fp8, tag="swt", name="swt")
                hk = KT // 2
                for h in range(2):
                    w_piece(swt, ot, h, hk)
                return swt

            ck4 = KT // 4
            swt0 = swp.tile([128, KT, 128], fp8, tag="swt", name="swt")
            swt1 = swp.tile([128, KT, 128], fp8, tag="swt", name="swt")
            w_piece(swt0, 0, 0, ck4)
            w_piece(swt1, 1, 0, ck4)
            x_dma(0)
            for h in range(1, 4):
                w_piece(swt0, 0, h, ck4)
                w_piece(swt1, 1, h, ck4)
            for kp in range(1, KP):
                x_dma(kp)
            nc.sync.dma_start(out=gbt[:], in_=gb[:])
            nc.gpsimd.collective_compute(
                "AllGather",
                mybir.AluOpType.bypass,
                replica_groups=[list(range(N_CORES))],
                ins=[ccd_in[:]],
                outs=[ccd_out[:]],
            )

            assert GS[0] >= 3, "fused ot0/ot1 pass assumes both in group 0"
            pss01 = mm_fused01([swt0, swt1])
            swt_next = w_dma(2)
            rdma_prep(0)
            aggrs = [mm_drain(0, pss01[0])[0], mm_drain(1, pss01[1])[0]]

            # post(q) anchors: far enough after group q's exchange fired
            # that the peers' writes have certainly landed even with
            # inter-core skew (post(0) sits 5+ tiles past group 0's
            # trigger). A blocked DVE wait at tile t delays only tile
            # t+2's matmuls (psp bufs=2), so post(1)@OT-2 can never gate
            # the PE at all. norm(q) follows post(q).
            post_at = {OT - 4: 0, OT - 2: 1}

            last_tile_psums = None
            norm1_last = None
            for q in range(NSPLIT):
                for ot in range(max(GO[q], 2), GO[q] + GS[q]):
                    swt = swt_next
                    if ot + 1 < OT:
                        swt_next = w_dma(ot + 1)
                    is_last = ot == OT - 1
                    aggr, psums = mm_tile(ot, swt, defer_casts=is_last)
                    aggrs.append(aggr)
                    if is_last:
                        last_tile_psums = psums
                    if ot == GO[q] + GS[q] - 1:
                        own = stats_pre(q)
                        if q + 1 < NSPLIT:
                            rdma_prep(q + 1)
                    pq = post_at.get(ot)
                    if pq is not None:
                        stats_post(pq, anchor=aggrs[ot])
                        norm1_last = norm_group(pq)

            # Tail: the deferred last-tile casts fill the exchange
            # latency; the final wait is ordered after them and after the
            # previous group's normalize so nothing queues behind it.
            casts = []
            for bt in range(BT):
                casts.append(nc.vector.tensor_copy(
                    yTt[:, OT - 1, ts(bt, NB)], last_tile_psums[bt][:]
                ))
            pre = [casts[-1], own]
            if norm1_last is not None:
                pre.append(norm1_last)
            stats_post(NSPLIT - 1, pre_deps=pre)
            norm_group(NSPLIT - 1)

    nc.finalize()
    return nc


def shard_inputs(x, w, gamma, beta, kb=KB_SHARD, ko=KO_SHARD):
    B, IN = x.shape
    OUT = w.shape[0]
    Bc = B // kb
    OUTc = OUT // ko
    KT, OT = IN // 128, OUTc // 128
    e4 = ml_dtypes.float8_e4m3
    # Host-computed sign bytes (exact): x -> +-0.5 (0x30/0xB0),
    # w -> +-1 (0x38/0xB8). signbit(+0.) is False, so sign(0) maps to +,
    # which differs from sign()'s 0 on a measure-zero set of fp32 randn.
    xs = np.where(np.signbit(x), 0xB0, 0x30).astype(np.uint8)
    ws = np.where(np.signbit(w), 0xB8, 0x38).astype(np.uint8)
    xts = []
    for ib in range(kb):
        xts.append(np.ascontiguousarray(
            xs[ib * Bc : (ib + 1) * Bc].T
        ).view(e4))
    wgs = []
    for io in range(ko):
        wsh = ws[io * OUTc : (io + 1) * OUTc]
        w2 = np.ascontiguousarray(
            wsh.reshape(OT, 128, KT, 128).transpose(0, 3, 2, 1)
        ).view(e4)
        gbp = np.ascontiguousarray(np.stack(
            [gamma[io * OUTc : (io + 1) * OUTc].reshape(OT, 128).T,
             beta[io * OUTc : (io + 1) * OUTc].reshape(OT, 128).T],
            axis=1,
        )).astype(np.float32)
        wgs.append((w2, gbp))
    in_maps = []
    for c in range(kb * ko):
        io, ib = c // kb, c % kb
        in_maps.append({"xt": xts[ib], "w2": wgs[io][0], "gb": wgs[io][1]})
    return in_maps


_NC_CACHE = {}


def kernel(x, w, gamma, beta):
    x = np.asarray(x)
    w = np.asarray(w)
    gamma = np.asarray(gamma)
    beta = np.asarray(beta)
    B, IN = x.shape
    OUT = w.shape[0]

    key = (B, IN, OUT)
    if key not in _NC_CACHE:
        _NC_CACHE[key] = build(B, IN, OUT)
    nc = _NC_CACHE[key]

    in_maps = shard_inputs(x, w, gamma, beta)
    res = run_bass_kernel_spmd(nc, in_maps, list(range(N_CORES)))
    Bc, OUTc = B // KB_SHARD, OUT // KO_SHARD
    out = np.empty((B, OUT), np.float32)
    for c in range(N_CORES):
        io, ib = c // KB_SHARD, c % KB_SHARD
        out[ib * Bc : (ib + 1) * Bc, io * OUTc : (io + 1) * OUTc] = (
            res.results[c]["yt"].T.astype(np.float32)
        )
    return out


if __name__ == "__main__":
    rng = np.random.default_rng(0)
    B, IN, OUT = 8192, 4096, 4096
    x = rng.standard_normal((B, IN)).astype(np.float32)
    w = rng.standard_normal((OUT, IN)).astype(np.float32)
    gamma = np.ones(OUT, np.float32)
    beta = np.zeros(OUT, np.float32)
    out = kernel(x, w, gamma, beta)
    print(out.shape, out.dtype)
